# revision 41
# baseline (speedup 1.0000x reference)
"""Trainium2 Bass kernel for an AttentionBlock (single-head spatial self-attention).

Reference computation (per batch b):
    xf = x[b] reshaped [N=HW, C]                 (x[b] stored as [C, N])
    q = xf @ wq + bq     [N, 64]
    k = xf @ wk + bk     [N, 64]
    v = xf @ wv + bv     [N, 512]
    att = softmax(q @ k^T) @ v                   [N, 512]
    y[b] = gamma * att + x[b]

Sharding: 8 cores = 4 batches x 2 query-halves. Each core holds full x[b]
(for K/V) plus its query half, computes attention for 2048 query rows, and
writes a [512, 2048] output slab. Host assembles the full output.

Kernel strategy (per core):
  - kT/qT computed directly in [d, n] layout with d duplicated across both
    SBUF partition halves (lhsT = [wk|wk]) -> enables row-tiled packing of
    the d=64-contraction E^T matmuls (two concurrent 64-row matmuls).
  - E^T = K Q^T computed with keys on partitions; 4 key-tiles' worth of
    E^T land in one [128,1024] PSUM tile so a single wide exp instruction
    amortizes ACT's ~352-cycle fixed cost; exp output (bf16) feeds the
    A @ V matmul directly as lhsT (no transposes in the softmax path).
  - softmax denominator comes free from a ones-column prepended to V
    (rhs chunks [257, 256] instead of [512]).
  - No max subtraction: E in [-63, 65] so exp stays in fp32/bf16 range.
  - q/k/v matmuls in float32r (1 cycle/row on PE; DMA into f32r tiles
    rounds to f32r, which the BIR verifier requires); P/V in bf16.
  - attention out is scaled by 1/denom, transposed back to [C, N] via PE
    transpose, then y = (gamma*att + gamma*bv) + x via ACT scale/bias + DVE add.
  - x[b] is host-rotated per core so its query half is always columns 0..2047:
    qT reads the same streamed x chunks as kT/V (no duplicate DMA; softmax
    and the attention sum are invariant to key order).
  - emission order software-pipelines chunks: E^T quads for chunk c+1 and
    transposes for chunk c-1 are interleaved between the A@V slices of
    chunk c so PE never waits on ACT exp.
"""

import numpy as np

import concourse.bacc as bacc
import concourse.mybir as mybir
import concourse.tile as tile
from concourse import bass_utils
from concourse.masks import make_identity

P = 128
C = 512
N = 4096
NQ = 2048  # queries per core
D = 64
CT = C // P  # 4 contraction slices
NT = N // P  # 32 key tiles
QCH = 8  # query chunks per core
QC = NQ // QCH  # 256 queries per chunk
QT = QC // P  # 2 query tiles per chunk
NG = NT // 4  # 8 quad-groups of key tiles

F32 = mybir.dt.float32
F32R = mybir.dt.float32r
BF16 = mybir.dt.bfloat16
AF = mybir.ActivationFunctionType
ALU = mybir.AluOpType

_CACHED_NC = None


def _build_kernel():
    nc = bacc.Bacc("TRN2", debug=False)

    xkv_d = nc.dram_tensor("xkv", [C, N], F32R, kind="ExternalInput").ap()
    xqf_d = nc.dram_tensor("xqf", [C, NQ], F32, kind="ExternalInput").ap()
    wkk_d = nc.dram_tensor("wkk", [C, P], F32R, kind="ExternalInput").ap()
    wqq_d = nc.dram_tensor("wqq", [C, P], F32R, kind="ExternalInput").ap()
    wv_d = nc.dram_tensor("wv", [C, C], F32R, kind="ExternalInput").ap()
    aux_d = nc.dram_tensor("aux", [P, 8], F32, kind="ExternalInput").ap()
    y_d = nc.dram_tensor("y", [C, NQ], F32, kind="ExternalOutput").ap()

    with tile.TileContext(nc) as tc:
        _body(nc, tc, xkv_d, xqf_d, wkk_d, wqq_d, wv_d, aux_d, y_d)
    nc.compile()
    return nc


def _body(nc, tc, xkv_d, xqf_d, wkk_d, wqq_d, wv_d, aux_d, y_d):
    from contextlib import ExitStack

    ctx = ExitStack()
    with ctx:
        const_pool = ctx.enter_context(tc.tile_pool(name="const", bufs=1))
        kq_pool = ctx.enter_context(tc.tile_pool(name="kq", bufs=1))
        vaug_pool = ctx.enter_context(tc.tile_pool(name="vaug", bufs=1))
        us_pool = ctx.enter_context(tc.tile_pool(name="us", bufs=2))
        y_pool = ctx.enter_context(tc.tile_pool(name="ypool", bufs=4))
        xr_pool = ctx.enter_context(tc.tile_pool(name="xrpool", bufs=4))
        small_pool = ctx.enter_context(tc.tile_pool(name="small", bufs=2))

        # ---- constants / weights ----
        wkk_t = const_pool.tile([P, CT * P], F32R)
        wqq_t = const_pool.tile([P, CT * P], F32R)
        wv_t = const_pool.tile([P, CT * C], F32R)
        aux_t = const_pool.tile([P, 8], F32)
        ident = const_pool.tile([P, P], BF16)
        # only wkk is needed before the first matmul; the rest of the
        # constant loads are emitted inside A/B iteration 0 (after chunk 0's
        # x DMA) so they don't delay the kernel start
        nc.scalar.dma_start(wkk_t[:, 0:P], wkk_d[0:P, :])
        nc.scalar.dma_start(aux_t[:], aux_d[:])
        nc.scalar.dma_start(
            wkk_t[:, P:].rearrange("p (ks c) -> p ks c", ks=CT - 1),
            wkk_d[P:, :].rearrange("(ks p) c -> p ks c", ks=CT - 1),
        )

        def load_late_consts():
            nc.gpsimd.dma_start(wv_t[:, 0:C], wv_d[0:P, :])
            nc.gpsimd.dma_start(
                wv_t[:, C:].rearrange("p (ks c) -> p ks c", ks=CT - 1),
                wv_d[P:, :].rearrange("(ks p) c -> p ks c", ks=CT - 1),
            )
            nc.scalar.dma_start(wqq_t[:].rearrange("p (ks c) -> p ks c", ks=CT), wqq_d.rearrange("(ks p) c -> p ks c", ks=CT))
        make_identity(nc, ident[:])

        # ---- persistent intermediates ----
        ktd_t = kq_pool.tile([P, N], BF16)
        qtd_t = kq_pool.tile([P, NQ], BF16)
        # v_aug tiles: [128 keys, 1 + 512] bf16, col 0 = ones
        vaug = [vaug_pool.tile([P, 1 + C], BF16, name=f"vaug{nt}", tag=f"vaug{nt}") for nt in range(NT)]

        pt_pool = ctx.enter_context(tc.tile_pool(name="pt", bufs=2))
        ps_e_pool = ctx.enter_context(tc.tile_pool(name="pse", bufs=1, space="PSUM"))
        # pt quad tiles: cols EPOS[j]*QC.. hold exp(E^T) for key tile 4g+j.
        # One tile per (chunk, g), allocated at emission so the per-tag
        # double buffer rotates: exp for chunk c+1 writes the other slot
        # while chunk c's A@V still reads the current one.
        # pt_cur[g] = quads consumed by the current chunk's A@V;
        # pt_next[g] = quads being produced for the next chunk.
        pt_cur = [None] * NG
        pt_next = [None] * NG
        # key tile 4g+j lives at column block EPOS[j]*QC of pse/pt quad
        # (pair members j=0/1 and j=2/3 run concurrently in different
        # row groups -> must write different PSUM banks: blocks 0,2|1,3)
        EPOS = (0, 2, 1, 3)

        def emit_e_quad(qc, g):
            """E^T + exp for key tiles 4g..4g+3, query chunk qc."""
            q0 = qc * QC
            pse = ps_e_pool.tile([P, 4 * QC], F32, name="pse", tag="e")
            for j in range(4):
                kt = 4 * g + j
                half = j % 2
                nc.tensor.matmul(
                    pse[:, EPOS[j] * QC : (EPOS[j] + 1) * QC],
                    ktd_t[half * D : (half + 1) * D, kt * P : (kt + 1) * P],
                    qtd_t[half * D : (half + 1) * D, q0 : q0 + QC],
                    start=True,
                    stop=True,
                    tile_position=(half * D, 0),
                )
            ptg = pt_pool.tile([P, 4 * QC], BF16, name=f"pt{g}", tag=f"pt{g}")
            nc.scalar.activation(ptg[:], pse[:], AF.Exp)
            pt_next[g] = ptg

        # E^T(chunk 0) quads interleaved into the A/B stream once their
        # ktd chunk (g) and qtd chunk 0 are available
        E0_AFTER_CH = {2: [0], 3: [1], 4: [2, 3], 5: [4], 6: [5, 6], 7: [7]}

        with tc.tile_pool(name="xpool", bufs=3) as x_pool, \
             tc.tile_pool(name="pskq", bufs=2, space="PSUM") as ps_kq_pool, \
             tc.tile_pool(name="psv", bufs=2, space="PSUM") as ps_v_pool:
            # ---- phases A+B streamed over 512-col chunks of x[b]:
            # kT chunk (+dup) and V for the 4 key tiles of the chunk
            xc_tiles = []
            for ch in range(N // 512):
                xc = x_pool.tile([P, CT * 512], F32R, name="xc", tag="xc")
                xc_tiles.append(xc)
                # chunk 0 streams in 256-col pieces so the first matmul can
                # start after only 128KB has landed
                nsub = 2 if ch == 0 else 1
                w = 512 // nsub
                for ks in range(CT):
                    for sb in range(nsub):
                        nc.sync.dma_start(
                            xc[:, ks * 512 + sb * w : ks * 512 + (sb + 1) * w],
                            xkv_d[ks * P : (ks + 1) * P, ch * 512 + sb * w : ch * 512 + (sb + 1) * w],
                        )
                if ch == 0:
                    load_late_consts()
                ps = ps_kq_pool.tile([P, 512], F32, name="pskq", tag="kq")
                for sb in range(nsub):
                    for ks in range(CT):
                        nc.tensor.matmul(
                            ps[:, sb * w : (sb + 1) * w],
                            wkk_t[:, ks * P : (ks + 1) * P],
                            xc[:, ks * 512 + sb * w : ks * 512 + sb * w + w],
                            start=(ks == 0),
                            stop=(ks == CT - 1),
                        )
                nc.vector.tensor_scalar_add(ktd_t[:, ch * 512 : (ch + 1) * 512], ps[:], aux_t[:, 0:1])
                for j in range(4):
                    nt = ch * 4 + j
                    psv = ps_v_pool.tile([P, C], F32, name="psv", tag="v")
                    for ks in range(CT):
                        nc.tensor.matmul(
                            psv[:],
                            xc[:, ks * 512 + j * P : ks * 512 + (j + 1) * P],
                            wv_t[:, ks * C : (ks + 1) * C],
                            start=(ks == 0),
                            stop=(ks == CT - 1),
                        )
                    nc.vector.memset(vaug[nt][:, 0:1], 1.0)
                    nc.vector.tensor_copy(vaug[nt][:, 1 : 1 + C], psv[:])
                # qT-gen (duplicated across partition halves) for query slab
                # qh = ch-1, read from the previous iteration's xc chunk
                # (x[b] is host-rotated so queries are chunks 0..3)
                if 1 <= ch <= NQ // 512:
                    qh = ch - 1
                    xq_src = xc_tiles[qh]
                    ps = ps_kq_pool.tile([P, 512], F32, name="pskq2", tag="kq")
                    for ks in range(CT):
                        nc.tensor.matmul(
                            ps[:],
                            wqq_t[:, ks * P : (ks + 1) * P],
                            xq_src[:, ks * 512 : (ks + 1) * 512],
                            start=(ks == 0),
                            stop=(ks == CT - 1),
                        )
                    nc.vector.tensor_scalar_add(qtd_t[:, qh * 512 : (qh + 1) * 512], ps[:], aux_t[:, 1:2])
                for g in E0_AFTER_CH.get(ch, []):
                    emit_e_quad(0, g)

        # ---- phase C: attention, software-pipelined query chunks of 256 ----
        with tc.tile_pool(name="psu", bufs=1, space="PSUM") as ps_u_pool, \
             tc.tile_pool(name="pst", bufs=2, space="PSUM") as ps_t_pool:
            def emit_u_slice(qc, qt, kt, psa, psb):
                g, j = divmod(kt, 4)
                jp = EPOS[j]
                lhs = pt_cur[g][:, jp * QC + qt * P : jp * QC + (qt + 1) * P]
                nc.tensor.matmul(psa[:], lhs, vaug[kt][:, 0:257], start=(kt == 0), stop=(kt == NT - 1))
                nc.tensor.matmul(psb[:], lhs, vaug[kt][:, 257:513], start=(kt == 0), stop=(kt == NT - 1))

            def emit_u_epilogue(qc, qt, psa, psb, us_ts):
                inv = small_pool.tile([P, 1], F32, name="inv", tag="inv")
                nc.vector.reciprocal(inv[:], psa[:, 0:1])
                us_t = us_pool.tile([P, C], BF16, name=f"us{qt}", tag=f"us{qt}")
                nc.vector.tensor_scalar_mul(us_t[:, 0:256], psa[:, 1:257], inv[:])
                nc.vector.tensor_scalar_mul(us_t[:, 256:512], psb[:, 0:256], inv[:])
                us_ts.append(us_t)

            def emit_tail_ct(qc, ct, us_ts):
                """Transpose + gamma/residual + store for one c-tile of chunk qc."""
                q0 = qc * QC
                pst = ps_t_pool.tile([P, QC], BF16, name="pst", tag="t")
                for qt in range(QT):
                    nc.tensor.transpose(
                        pst[:, qt * P : (qt + 1) * P],
                        us_ts[qt][:, ct * P : (ct + 1) * P],
                        ident[:],
                    )
                xr = xr_pool.tile([P, QC], F32, name="xr", tag="xr")
                nc.sync.dma_start(xr[:], xqf_d[ct * P : (ct + 1) * P, q0 : q0 + QC])
                yt = y_pool.tile([P, QC], F32, name="yt", tag="y")
                nc.scalar.activation(
                    yt[:], pst[:], AF.Identity,
                    bias=aux_t[:, 3 + ct : 4 + ct], scale=aux_t[:, 2:3],
                )
                nc.vector.tensor_add(yt[:], yt[:], xr[:])
                nc.sync.dma_start(y_d[ct * P : (ct + 1) * P, q0 : q0 + QC], yt[:])

            prev_us = None  # us tiles of chunk c-1
            for qc in range(QCH):
                for g in range(NG):
                    pt_cur[g] = pt_next[g]
                    pt_next[g] = None
                us_ts = []
                psu = []
                for qt in range(QT):
                    psa = ps_u_pool.tile([P, 257], F32, name=f"psua{qt}", tag=f"ua{qt}")
                    psb = ps_u_pool.tile([P, 256], F32, name=f"psub{qt}", tag=f"ub{qt}")
                    psu.append((psa, psb))
                for i in range(2 * NT):  # 64 U slices
                    qt, kt = divmod(i, NT)
                    # interleave: one E^T quad for chunk qc+1 every 8 slices
                    if i % 8 == 4 and qc + 1 < QCH:
                        emit_e_quad(qc + 1, i // 8)
                    # interleave: tail work for chunk qc-1 during early slices
                    if prev_us is not None and i in (2, 10, 18, 26):
                        emit_tail_ct(qc - 1, i // 8, prev_us)
                    emit_u_slice(qc, qt, kt, *psu[qt])
                    if kt == NT - 1:
                        emit_u_epilogue(qc, qt, *psu[qt], us_ts)
                prev_us = us_ts
            for ct in range(CT):
                emit_tail_ct(QCH - 1, ct, prev_us)


def kernel(x, wq, bq, wk, bk, wv, bv, gamma, **_ignored):
    global _CACHED_NC
    x = np.ascontiguousarray(np.asarray(x, dtype=np.float32))
    wq = np.asarray(wq, dtype=np.float32)
    wk = np.asarray(wk, dtype=np.float32)
    wv = np.ascontiguousarray(np.asarray(wv, dtype=np.float32))
    bq = np.asarray(bq, dtype=np.float32)
    bk = np.asarray(bk, dtype=np.float32)
    bv = np.asarray(bv, dtype=np.float32)
    gamma = np.asarray(gamma, dtype=np.float32).reshape(())

    B = x.shape[0]
    assert x.shape == (B, C, 64, 64) and B * 2 == 8

    wkk = np.ascontiguousarray(np.concatenate([wk, wk], axis=1))  # [C, 128]
    wqq = np.ascontiguousarray(np.concatenate([wq, wq], axis=1))  # [C, 128]
    aux = np.zeros((P, 8), dtype=np.float32)
    aux[:, 0] = np.concatenate([bk, bk])
    aux[:, 1] = np.concatenate([bq, bq])
    aux[:, 2] = gamma
    gbv = gamma * bv
    for ct in range(CT):
        aux[:, 3 + ct] = gbv[ct * P : (ct + 1) * P]

    if _CACHED_NC is None:
        _CACHED_NC = _build_kernel()
    nc = _CACHED_NC

    xf = x.reshape(B, C, N)
    in_maps = []
    for core in range(8):
        b, h = divmod(core, 2)
        xqh = np.ascontiguousarray(xf[b][:, h * NQ : (h + 1) * NQ])
        xrot = np.ascontiguousarray(np.roll(xf[b], -h * NQ, axis=1))
        in_maps.append(
            {
                "xkv": xrot,
                "xqf": xqh,
                "wkk": wkk,
                "wqq": wqq,
                "wv": wv,
                "aux": aux,
            }
        )

    res = bass_utils.run_bass_kernel_spmd(nc, in_maps, core_ids=list(range(8)))
    if res.exec_time_ns is not None:
        print(f"HW exec time: {res.exec_time_ns} ns", flush=True)
        if res.instructions_and_trace is not None:
            print(f"trace: {res.instructions_and_trace[1]}", flush=True)

    y = np.empty((B, C, N), dtype=np.float32)
    for core in range(8):
        b, h = divmod(core, 2)
        y[b][:, h * NQ : (h + 1) * NQ] = res.results[core]["y"]
    return y.reshape(B, C, 64, 64)


# revision 44
# speedup vs baseline: 1.0017x; 1.0017x over previous
"""Trainium2 Bass kernel for an AttentionBlock (single-head spatial self-attention).

Reference computation (per batch b):
    xf = x[b] reshaped [N=HW, C]                 (x[b] stored as [C, N])
    q = xf @ wq + bq     [N, 64]
    k = xf @ wk + bk     [N, 64]
    v = xf @ wv + bv     [N, 512]
    att = softmax(q @ k^T) @ v                   [N, 512]
    y[b] = gamma * att + x[b]

Sharding: 8 cores = 4 batches x 2 query-halves. Each core holds full x[b]
(for K/V) plus its query half, computes attention for 2048 query rows, and
writes a [512, 2048] output slab. Host assembles the full output.

Kernel strategy (per core):
  - kT/qT computed directly in [d, n] layout with d duplicated across both
    SBUF partition halves (lhsT = [wk|wk]) -> enables row-tiled packing of
    the d=64-contraction E^T matmuls (two concurrent 64-row matmuls).
  - E^T = K Q^T computed with keys on partitions; 4 key-tiles' worth of
    E^T land in one [128,1024] PSUM tile so a single wide exp instruction
    amortizes ACT's ~352-cycle fixed cost; exp output (bf16) feeds the
    A @ V matmul directly as lhsT (no transposes in the softmax path).
  - softmax denominator comes free from a ones-column prepended to V
    (rhs chunks [257, 256] instead of [512]).
  - No max subtraction: E in [-63, 65] so exp stays in fp32/bf16 range.
  - q/k/v matmuls in float32r (1 cycle/row on PE; DMA into f32r tiles
    rounds to f32r, which the BIR verifier requires); P/V in bf16.
  - attention out is scaled by 1/denom, transposed back to [C, N] via PE
    transpose, then y = (gamma*att + gamma*bv) + x via ACT scale/bias + DVE add.
  - x[b] is host-rotated per core so its query half is always columns 0..2047:
    qT reads the same streamed x chunks as kT/V (no duplicate DMA; softmax
    and the attention sum are invariant to key order).
  - emission order software-pipelines chunks: E^T quads for chunk c+1 and
    transposes for chunk c-1 are interleaved between the A@V slices of
    chunk c so PE never waits on ACT exp.
"""

import numpy as np

import concourse.bacc as bacc
import concourse.mybir as mybir
import concourse.tile as tile
from concourse import bass_utils
from concourse.masks import make_identity

P = 128
C = 512
N = 4096
NQ = 2048  # queries per core
D = 64
CT = C // P  # 4 contraction slices
NT = N // P  # 32 key tiles
QCH = 8  # query chunks per core
QC = NQ // QCH  # 256 queries per chunk
QT = QC // P  # 2 query tiles per chunk
NG = NT // 4  # 8 quad-groups of key tiles

F32 = mybir.dt.float32
F32R = mybir.dt.float32r
BF16 = mybir.dt.bfloat16
AF = mybir.ActivationFunctionType
ALU = mybir.AluOpType

_CACHED_NC = None


def _build_kernel():
    nc = bacc.Bacc("TRN2", debug=False)

    xkv_d = nc.dram_tensor("xkv", [C, N], F32R, kind="ExternalInput").ap()
    xqf_d = nc.dram_tensor("xqf", [C, NQ], F32, kind="ExternalInput").ap()
    wkk_d = nc.dram_tensor("wkk", [C, P], F32R, kind="ExternalInput").ap()
    wqq_d = nc.dram_tensor("wqq", [C, P], F32R, kind="ExternalInput").ap()
    wv_d = nc.dram_tensor("wv", [C, C], F32R, kind="ExternalInput").ap()
    aux_d = nc.dram_tensor("aux", [P, 8], F32, kind="ExternalInput").ap()
    y_d = nc.dram_tensor("y", [C, NQ], F32, kind="ExternalOutput").ap()

    with tile.TileContext(nc) as tc:
        _body(nc, tc, xkv_d, xqf_d, wkk_d, wqq_d, wv_d, aux_d, y_d)
    nc.compile()
    return nc


def _body(nc, tc, xkv_d, xqf_d, wkk_d, wqq_d, wv_d, aux_d, y_d):
    from contextlib import ExitStack

    ctx = ExitStack()
    with ctx:
        const_pool = ctx.enter_context(tc.tile_pool(name="const", bufs=1))
        kq_pool = ctx.enter_context(tc.tile_pool(name="kq", bufs=1))
        vaug_pool = ctx.enter_context(tc.tile_pool(name="vaug", bufs=1))
        us_pool = ctx.enter_context(tc.tile_pool(name="us", bufs=2))
        y_pool = ctx.enter_context(tc.tile_pool(name="ypool", bufs=4))
        xr_pool = ctx.enter_context(tc.tile_pool(name="xrpool", bufs=4))
        small_pool = ctx.enter_context(tc.tile_pool(name="small", bufs=2))

        # ---- constants / weights ----
        wkk_t = const_pool.tile([P, CT * P], F32R)
        wqq_t = const_pool.tile([P, CT * P], F32R)
        wv_t = const_pool.tile([P, CT * C], F32R)
        aux_t = const_pool.tile([P, 8], F32)
        ident = const_pool.tile([P, P], BF16)
        # only wkk is needed before the first matmul; the rest of the
        # constant loads are emitted inside A/B iteration 0 (after chunk 0's
        # x DMA) so they don't delay the kernel start
        nc.scalar.dma_start(wkk_t[:, 0:P], wkk_d[0:P, :])
        nc.scalar.dma_start(aux_t[:], aux_d[:])
        nc.scalar.dma_start(
            wkk_t[:, P:].rearrange("p (ks c) -> p ks c", ks=CT - 1),
            wkk_d[P:, :].rearrange("(ks p) c -> p ks c", ks=CT - 1),
        )

        def load_late_consts():
            nc.gpsimd.dma_start(wv_t[:, 0:C], wv_d[0:P, :])
            nc.gpsimd.dma_start(
                wv_t[:, C:].rearrange("p (ks c) -> p ks c", ks=CT - 1),
                wv_d[P:, :].rearrange("(ks p) c -> p ks c", ks=CT - 1),
            )
            nc.scalar.dma_start(wqq_t[:].rearrange("p (ks c) -> p ks c", ks=CT), wqq_d.rearrange("(ks p) c -> p ks c", ks=CT))
        make_identity(nc, ident[:])

        # ---- persistent intermediates ----
        ktd_t = kq_pool.tile([P, N], BF16)
        qtd_t = kq_pool.tile([P, NQ], BF16)
        # v_aug tiles: [128 keys, 1 + 512] bf16, col 0 = ones
        vaug = [vaug_pool.tile([P, 1 + C], BF16, name=f"vaug{nt}", tag=f"vaug{nt}") for nt in range(NT)]

        pt_pool = ctx.enter_context(tc.tile_pool(name="pt", bufs=2))
        ps_e_pool = ctx.enter_context(tc.tile_pool(name="pse", bufs=1, space="PSUM"))
        # pt quad tiles: cols EPOS[j]*QC.. hold exp(E^T) for key tile 4g+j.
        # One tile per (chunk, g), allocated at emission so the per-tag
        # double buffer rotates: exp for chunk c+1 writes the other slot
        # while chunk c's A@V still reads the current one.
        # pt_cur[g] = quads consumed by the current chunk's A@V;
        # pt_next[g] = quads being produced for the next chunk.
        pt_cur = [None] * NG
        pt_next = [None] * NG
        # key tile 4g+j lives at column block EPOS[j]*QC of pse/pt quad
        # (pair members j=0/1 and j=2/3 run concurrently in different
        # row groups -> must write different PSUM banks: blocks 0,2|1,3)
        EPOS = (0, 2, 1, 3)

        def emit_e_quad(qc, g):
            """E^T + exp for key tiles 4g..4g+3, query chunk qc."""
            q0 = qc * QC
            pse = ps_e_pool.tile([P, 4 * QC], F32, name="pse", tag="e")
            for j in range(4):
                kt = 4 * g + j
                half = j % 2
                nc.tensor.matmul(
                    pse[:, EPOS[j] * QC : (EPOS[j] + 1) * QC],
                    ktd_t[half * D : (half + 1) * D, kt * P : (kt + 1) * P],
                    qtd_t[half * D : (half + 1) * D, q0 : q0 + QC],
                    start=True,
                    stop=True,
                    tile_position=(half * D, 0),
                )
            ptg = pt_pool.tile([P, 4 * QC], BF16, name=f"pt{g}", tag=f"pt{g}")
            nc.scalar.activation(ptg[:], pse[:], AF.Exp)
            pt_next[g] = ptg

        # E^T(chunk 0) quads interleaved into the A/B stream once their
        # ktd chunk (g) and qtd chunk 0 are available
        E0_AFTER_CH = {2: [0], 3: [1], 4: [2, 3], 5: [4], 6: [5, 6], 7: [7]}

        with tc.tile_pool(name="xpool", bufs=3) as x_pool, \
             tc.tile_pool(name="pskq", bufs=2, space="PSUM") as ps_kq_pool, \
             tc.tile_pool(name="psv", bufs=2, space="PSUM") as ps_v_pool:
            # ---- phases A+B streamed over 512-col chunks of x[b]:
            # kT chunk (+dup) and V for the 4 key tiles of the chunk
            xc_tiles = []
            for ch in range(N // 512):
                xc = x_pool.tile([P, CT * 512], F32R, name="xc", tag="xc")
                xc_tiles.append(xc)
                # chunk 0 streams in 256-col pieces so the first matmul can
                # start after only 128KB has landed
                nsub = 2 if ch == 0 else 1
                w = 512 // nsub
                for ks in range(CT):
                    for sb in range(nsub):
                        nc.sync.dma_start(
                            xc[:, ks * 512 + sb * w : ks * 512 + (sb + 1) * w],
                            xkv_d[ks * P : (ks + 1) * P, ch * 512 + sb * w : ch * 512 + (sb + 1) * w],
                        )
                if ch == 0:
                    load_late_consts()
                ps = ps_kq_pool.tile([P, 512], F32, name="pskq", tag="kq")
                for sb in range(nsub):
                    for ks in range(CT):
                        nc.tensor.matmul(
                            ps[:, sb * w : (sb + 1) * w],
                            wkk_t[:, ks * P : (ks + 1) * P],
                            xc[:, ks * 512 + sb * w : ks * 512 + sb * w + w],
                            start=(ks == 0),
                            stop=(ks == CT - 1),
                        )
                nc.vector.tensor_scalar_add(ktd_t[:, ch * 512 : (ch + 1) * 512], ps[:], aux_t[:, 0:1])
                for j in range(4):
                    nt = ch * 4 + j
                    psv = ps_v_pool.tile([P, C], F32, name="psv", tag="v")
                    for ks in range(CT):
                        nc.tensor.matmul(
                            psv[:],
                            xc[:, ks * 512 + j * P : ks * 512 + (j + 1) * P],
                            wv_t[:, ks * C : (ks + 1) * C],
                            start=(ks == 0),
                            stop=(ks == CT - 1),
                        )
                    nc.vector.memset(vaug[nt][:, 0:1], 1.0)
                    nc.vector.tensor_copy(vaug[nt][:, 1 : 1 + C], psv[:])
                # qT-gen (duplicated across partition halves) for query slab
                # qh = ch-1, read from the previous iteration's xc chunk
                # (x[b] is host-rotated so queries are chunks 0..3)
                if 1 <= ch <= NQ // 512:
                    qh = ch - 1
                    xq_src = xc_tiles[qh]
                    ps = ps_kq_pool.tile([P, 512], F32, name="pskq2", tag="kq")
                    for ks in range(CT):
                        nc.tensor.matmul(
                            ps[:],
                            wqq_t[:, ks * P : (ks + 1) * P],
                            xq_src[:, ks * 512 : (ks + 1) * 512],
                            start=(ks == 0),
                            stop=(ks == CT - 1),
                        )
                    nc.vector.tensor_scalar_add(qtd_t[:, qh * 512 : (qh + 1) * 512], ps[:], aux_t[:, 1:2])
                for g in E0_AFTER_CH.get(ch, []):
                    emit_e_quad(0, g)

        # ---- phase C: attention, software-pipelined query chunks of 256 ----
        with tc.tile_pool(name="psu", bufs=1, space="PSUM") as ps_u_pool, \
             tc.tile_pool(name="pst", bufs=2, space="PSUM") as ps_t_pool:
            def emit_u_slice(qc, qt, kt, psa, psb):
                g, j = divmod(kt, 4)
                jp = EPOS[j]
                lhs = pt_cur[g][:, jp * QC + qt * P : jp * QC + (qt + 1) * P]
                nc.tensor.matmul(psa[:], lhs, vaug[kt][:, 0:257], start=(kt == 0), stop=(kt == NT - 1))
                nc.tensor.matmul(psb[:], lhs, vaug[kt][:, 257:513], start=(kt == 0), stop=(kt == NT - 1))

            def emit_u_epilogue(qc, qt, psa, psb, us_ts):
                inv = small_pool.tile([P, 1], F32, name="inv", tag="inv")
                nc.vector.reciprocal(inv[:], psa[:, 0:1])
                us_t = us_pool.tile([P, C], BF16, name=f"us{qt}", tag=f"us{qt}")
                nc.vector.tensor_scalar_mul(us_t[:, 0:256], psa[:, 1:257], inv[:])
                nc.vector.tensor_scalar_mul(us_t[:, 256:512], psb[:, 0:256], inv[:])
                us_ts.append(us_t)

            last_y2 = {}

            def emit_tail_ct(qc, ct, us_ts):
                """Transpose + gamma/residual + store for one c-tile of chunk qc.
                For the final chunk the stores are batched in c-tile pairs:
                the four per-tile stores would otherwise serialize their
                ~1.4us DMA fixed cost into the kernel tail."""
                q0 = qc * QC
                last = qc == QCH - 1
                pst = ps_t_pool.tile([P, QC], BF16, name="pst", tag="t")
                for qt in range(QT):
                    nc.tensor.transpose(
                        pst[:, qt * P : (qt + 1) * P],
                        us_ts[qt][:, ct * P : (ct + 1) * P],
                        ident[:],
                    )
                xr = xr_pool.tile([P, QC], F32, name="xr", tag="xr")
                # final chunk: residual loads go on the scalar queue so they
                # don't queue behind the tail y-stores on sync
                (nc.scalar if last else nc.sync).dma_start(
                    xr[:], xqf_d[ct * P : (ct + 1) * P, q0 : q0 + QC]
                )
                if last:
                    if ct % 2 == 0:
                        last_y2[ct // 2] = y_pool.tile([P, 2 * QC], F32, name="y2", tag="y2", bufs=2)
                    yt = last_y2[ct // 2][:, (ct % 2) * QC : (ct % 2 + 1) * QC]
                else:
                    yt = y_pool.tile([P, QC], F32, name="yt", tag="y")[:]
                nc.scalar.activation(
                    yt, pst[:], AF.Identity,
                    bias=aux_t[:, 3 + ct : 4 + ct], scale=aux_t[:, 2:3],
                )
                nc.vector.tensor_add(yt, yt, xr[:])
                if last:
                    if ct % 2 == 1:
                        pair = ct // 2
                        nc.sync.dma_start(
                            y_d[pair * 2 * P : (pair + 1) * 2 * P, q0 : q0 + QC].rearrange(
                                "(ct p) q -> p ct q", ct=2
                            ),
                            last_y2[pair][:].rearrange("p (ct q) -> p ct q", ct=2),
                        )
                else:
                    nc.sync.dma_start(y_d[ct * P : (ct + 1) * P, q0 : q0 + QC], yt)

            prev_us = None  # us tiles of chunk c-1
            for qc in range(QCH):
                for g in range(NG):
                    pt_cur[g] = pt_next[g]
                    pt_next[g] = None
                us_ts = []
                psu = []
                for qt in range(QT):
                    psa = ps_u_pool.tile([P, 257], F32, name=f"psua{qt}", tag=f"ua{qt}")
                    psb = ps_u_pool.tile([P, 256], F32, name=f"psub{qt}", tag=f"ub{qt}")
                    psu.append((psa, psb))
                for i in range(2 * NT):  # 64 U slices
                    qt, kt = divmod(i, NT)
                    # interleave: one E^T quad for chunk qc+1 every 8 slices
                    if i % 8 == 4 and qc + 1 < QCH:
                        emit_e_quad(qc + 1, i // 8)
                    # interleave: tail work for chunk qc-1 during early slices
                    if prev_us is not None and i in (2, 10, 18, 26):
                        emit_tail_ct(qc - 1, i // 8, prev_us)
                    emit_u_slice(qc, qt, kt, *psu[qt])
                    if kt == NT - 1:
                        emit_u_epilogue(qc, qt, *psu[qt], us_ts)
                prev_us = us_ts
            for ct in range(CT):
                emit_tail_ct(QCH - 1, ct, prev_us)


def kernel(x, wq, bq, wk, bk, wv, bv, gamma, **_ignored):
    global _CACHED_NC
    x = np.ascontiguousarray(np.asarray(x, dtype=np.float32))
    wq = np.asarray(wq, dtype=np.float32)
    wk = np.asarray(wk, dtype=np.float32)
    wv = np.ascontiguousarray(np.asarray(wv, dtype=np.float32))
    bq = np.asarray(bq, dtype=np.float32)
    bk = np.asarray(bk, dtype=np.float32)
    bv = np.asarray(bv, dtype=np.float32)
    gamma = np.asarray(gamma, dtype=np.float32).reshape(())

    B = x.shape[0]
    assert x.shape == (B, C, 64, 64) and B * 2 == 8

    wkk = np.ascontiguousarray(np.concatenate([wk, wk], axis=1))  # [C, 128]
    wqq = np.ascontiguousarray(np.concatenate([wq, wq], axis=1))  # [C, 128]
    aux = np.zeros((P, 8), dtype=np.float32)
    aux[:, 0] = np.concatenate([bk, bk])
    aux[:, 1] = np.concatenate([bq, bq])
    aux[:, 2] = gamma
    gbv = gamma * bv
    for ct in range(CT):
        aux[:, 3 + ct] = gbv[ct * P : (ct + 1) * P]

    if _CACHED_NC is None:
        _CACHED_NC = _build_kernel()
    nc = _CACHED_NC

    xf = x.reshape(B, C, N)
    in_maps = []
    for core in range(8):
        b, h = divmod(core, 2)
        xqh = np.ascontiguousarray(xf[b][:, h * NQ : (h + 1) * NQ])
        xrot = np.ascontiguousarray(np.roll(xf[b], -h * NQ, axis=1))
        in_maps.append(
            {
                "xkv": xrot,
                "xqf": xqh,
                "wkk": wkk,
                "wqq": wqq,
                "wv": wv,
                "aux": aux,
            }
        )

    res = bass_utils.run_bass_kernel_spmd(nc, in_maps, core_ids=list(range(8)))
    if res.exec_time_ns is not None:
        print(f"HW exec time: {res.exec_time_ns} ns", flush=True)
        if res.instructions_and_trace is not None:
            print(f"trace: {res.instructions_and_trace[1]}", flush=True)

    y = np.empty((B, C, N), dtype=np.float32)
    for core in range(8):
        b, h = divmod(core, 2)
        y[b][:, h * NQ : (h + 1) * NQ] = res.results[core]["y"]
    return y.reshape(B, C, 64, 64)


# revision 46
# speedup vs baseline: 1.0034x; 1.0017x over previous
"""Trainium2 Bass kernel for an AttentionBlock (single-head spatial self-attention).

Reference computation (per batch b):
    xf = x[b] reshaped [N=HW, C]                 (x[b] stored as [C, N])
    q = xf @ wq + bq     [N, 64]
    k = xf @ wk + bk     [N, 64]
    v = xf @ wv + bv     [N, 512]
    att = softmax(q @ k^T) @ v                   [N, 512]
    y[b] = gamma * att + x[b]

Sharding: 8 cores = 4 batches x 2 query-halves. Each core holds full x[b]
(for K/V) plus its query half, computes attention for 2048 query rows, and
writes a [512, 2048] output slab. Host assembles the full output.

Kernel strategy (per core):
  - kT/qT computed directly in [d, n] layout with d duplicated across both
    SBUF partition halves (lhsT = [wk|wk]) -> enables row-tiled packing of
    the d=64-contraction E^T matmuls (two concurrent 64-row matmuls).
  - E^T = K Q^T computed with keys on partitions; 4 key-tiles' worth of
    E^T land in one [128,1024] PSUM tile so a single wide exp instruction
    amortizes ACT's ~352-cycle fixed cost; exp output (bf16) feeds the
    A @ V matmul directly as lhsT (no transposes in the softmax path).
  - softmax denominator comes free from a ones-column prepended to V
    (rhs chunks [257, 256] instead of [512]).
  - No max subtraction: E in [-63, 65] so exp stays in fp32/bf16 range.
  - q/k/v matmuls in float32r (1 cycle/row on PE; DMA into f32r tiles
    rounds to f32r, which the BIR verifier requires); P/V in bf16.
  - attention out is scaled by 1/denom, transposed back to [C, N] via PE
    transpose, then y = (gamma*att + gamma*bv) + x via ACT scale/bias + DVE add.
  - x[b] is host-rotated per core so its query half is always columns 0..2047:
    qT reads the same streamed x chunks as kT/V (no duplicate DMA; softmax
    and the attention sum are invariant to key order).
  - emission order software-pipelines chunks: E^T quads for chunk c+1 and
    transposes for chunk c-1 are interleaved between the A@V slices of
    chunk c so PE never waits on ACT exp.
"""

import numpy as np

import concourse.bacc as bacc
import concourse.mybir as mybir
import concourse.tile as tile
from concourse import bass_utils
from concourse.masks import make_identity

P = 128
C = 512
N = 4096
NQ = 2048  # queries per core
D = 64
CT = C // P  # 4 contraction slices
NT = N // P  # 32 key tiles
QCH = 8  # query chunks per core
QC = NQ // QCH  # 256 queries per chunk
QT = QC // P  # 2 query tiles per chunk
NG = NT // 4  # 8 quad-groups of key tiles

F32 = mybir.dt.float32
F32R = mybir.dt.float32r
BF16 = mybir.dt.bfloat16
AF = mybir.ActivationFunctionType
ALU = mybir.AluOpType

_CACHED_NC = None


def _build_kernel():
    nc = bacc.Bacc("TRN2", debug=False)

    xkv_d = nc.dram_tensor("xkv", [C, N], F32R, kind="ExternalInput").ap()
    xqf_d = nc.dram_tensor("xqf", [C, NQ], F32, kind="ExternalInput").ap()
    wkk_d = nc.dram_tensor("wkk", [C, P], F32R, kind="ExternalInput").ap()
    wqq_d = nc.dram_tensor("wqq", [C, P], F32R, kind="ExternalInput").ap()
    wv_d = nc.dram_tensor("wv", [C, C], F32R, kind="ExternalInput").ap()
    aux_d = nc.dram_tensor("aux", [P, 8], F32, kind="ExternalInput").ap()
    y_d = nc.dram_tensor("y", [C, NQ], F32, kind="ExternalOutput").ap()

    with tile.TileContext(nc) as tc:
        _body(nc, tc, xkv_d, xqf_d, wkk_d, wqq_d, wv_d, aux_d, y_d)
    nc.compile()
    return nc


def _body(nc, tc, xkv_d, xqf_d, wkk_d, wqq_d, wv_d, aux_d, y_d):
    from contextlib import ExitStack

    ctx = ExitStack()
    with ctx:
        const_pool = ctx.enter_context(tc.tile_pool(name="const", bufs=1))
        kq_pool = ctx.enter_context(tc.tile_pool(name="kq", bufs=1))
        vaug_pool = ctx.enter_context(tc.tile_pool(name="vaug", bufs=1))
        us_pool = ctx.enter_context(tc.tile_pool(name="us", bufs=2))
        y_pool = ctx.enter_context(tc.tile_pool(name="ypool", bufs=4))
        xr_pool = ctx.enter_context(tc.tile_pool(name="xrpool", bufs=4))
        small_pool = ctx.enter_context(tc.tile_pool(name="small", bufs=2))

        # ---- constants / weights ----
        wkk_t = const_pool.tile([P, CT * P], F32R)
        wqq_t = const_pool.tile([P, CT * P], F32R)
        wv_t = const_pool.tile([P, CT * C], F32R)
        aux_t = const_pool.tile([P, 8], F32)
        ident = const_pool.tile([P, P], BF16)
        # only wkk is needed before the first matmul; the rest of the
        # constant loads are emitted inside A/B iteration 0 (after chunk 0's
        # x DMA) so they don't delay the kernel start
        nc.scalar.dma_start(wkk_t[:, 0:P], wkk_d[0:P, :])
        nc.scalar.dma_start(aux_t[:], aux_d[:])
        nc.scalar.dma_start(
            wkk_t[:, P:].rearrange("p (ks c) -> p ks c", ks=CT - 1),
            wkk_d[P:, :].rearrange("(ks p) c -> p ks c", ks=CT - 1),
        )

        def load_wv_head():
            nc.gpsimd.dma_start(wv_t[:, 0:C], wv_d[0:P, :])

        def load_late_consts():
            nc.gpsimd.dma_start(
                wv_t[:, C:].rearrange("p (ks c) -> p ks c", ks=CT - 1),
                wv_d[P:, :].rearrange("(ks p) c -> p ks c", ks=CT - 1),
            )
            nc.scalar.dma_start(wqq_t[:].rearrange("p (ks c) -> p ks c", ks=CT), wqq_d.rearrange("(ks p) c -> p ks c", ks=CT))
        make_identity(nc, ident[:])

        # ---- persistent intermediates ----
        ktd_t = kq_pool.tile([P, N], BF16)
        qtd_t = kq_pool.tile([P, NQ], BF16)
        # v_aug tiles: [128 keys, 1 + 512] bf16, col 0 = ones
        vaug = [vaug_pool.tile([P, 1 + C], BF16, name=f"vaug{nt}", tag=f"vaug{nt}") for nt in range(NT)]

        pt_pool = ctx.enter_context(tc.tile_pool(name="pt", bufs=2))
        ps_e_pool = ctx.enter_context(tc.tile_pool(name="pse", bufs=1, space="PSUM"))
        # pt quad tiles: cols EPOS[j]*QC.. hold exp(E^T) for key tile 4g+j.
        # One tile per (chunk, g), allocated at emission so the per-tag
        # double buffer rotates: exp for chunk c+1 writes the other slot
        # while chunk c's A@V still reads the current one.
        # pt_cur[g] = quads consumed by the current chunk's A@V;
        # pt_next[g] = quads being produced for the next chunk.
        pt_cur = [None] * NG
        pt_next = [None] * NG
        # key tile 4g+j lives at column block EPOS[j]*QC of pse/pt quad
        # (pair members j=0/1 and j=2/3 run concurrently in different
        # row groups -> must write different PSUM banks: blocks 0,2|1,3)
        EPOS = (0, 2, 1, 3)

        def emit_e_quad(qc, g):
            """E^T + exp for key tiles 4g..4g+3, query chunk qc."""
            q0 = qc * QC
            pse = ps_e_pool.tile([P, 4 * QC], F32, name="pse", tag="e")
            for j in range(4):
                kt = 4 * g + j
                half = j % 2
                nc.tensor.matmul(
                    pse[:, EPOS[j] * QC : (EPOS[j] + 1) * QC],
                    ktd_t[half * D : (half + 1) * D, kt * P : (kt + 1) * P],
                    qtd_t[half * D : (half + 1) * D, q0 : q0 + QC],
                    start=True,
                    stop=True,
                    tile_position=(half * D, 0),
                )
            ptg = pt_pool.tile([P, 4 * QC], BF16, name=f"pt{g}", tag=f"pt{g}")
            nc.scalar.activation(ptg[:], pse[:], AF.Exp)
            pt_next[g] = ptg

        # E^T(chunk 0) quads interleaved into the A/B stream once their
        # ktd chunk (g) and qtd chunk 0 are available
        E0_AFTER_CH = {2: [0], 3: [1], 4: [2, 3], 5: [4], 6: [5, 6], 7: [7]}

        with tc.tile_pool(name="xpool", bufs=3) as x_pool, \
             tc.tile_pool(name="pskq", bufs=2, space="PSUM") as ps_kq_pool, \
             tc.tile_pool(name="psv", bufs=2, space="PSUM") as ps_v_pool:
            # ---- phases A+B streamed over 512-col chunks of x[b]:
            # kT chunk (+dup) and V for the 4 key tiles of the chunk
            xc_tiles = []
            for ch in range(N // 512):
                xc = x_pool.tile([P, CT * 512], F32R, name="xc", tag="xc")
                xc_tiles.append(xc)
                # chunk 0 streams in 256-col pieces so the first matmul can
                # start after only 128KB has landed
                nsub = 2 if ch == 0 else 1
                w = 512 // nsub
                # sb-major: the first kq sub-group needs the sb0 piece of
                # every ks, so those must land first
                for sb in range(nsub):
                    for ks in range(CT):
                        nc.sync.dma_start(
                            xc[:, ks * 512 + sb * w : ks * 512 + (sb + 1) * w],
                            xkv_d[ks * P : (ks + 1) * P, ch * 512 + sb * w : ch * 512 + (sb + 1) * w],
                        )
                    if ch == 0 and sb == 0:
                        load_wv_head()
                if ch == 0:
                    load_late_consts()
                ps = ps_kq_pool.tile([P, 512], F32, name="pskq", tag="kq")

                def emit_kq_group(sb):
                    for ks in range(CT):
                        nc.tensor.matmul(
                            ps[:, sb * w : (sb + 1) * w],
                            wkk_t[:, ks * P : (ks + 1) * P],
                            xc[:, ks * 512 + sb * w : ks * 512 + sb * w + w],
                            start=(ks == 0),
                            stop=(ks == CT - 1),
                        )

                def emit_v_tile(j):
                    nt = ch * 4 + j
                    psv = ps_v_pool.tile([P, C], F32, name="psv", tag="v")
                    for ks in range(CT):
                        nc.tensor.matmul(
                            psv[:],
                            xc[:, ks * 512 + j * P : ks * 512 + (j + 1) * P],
                            wv_t[:, ks * C : (ks + 1) * C],
                            start=(ks == 0),
                            stop=(ks == CT - 1),
                        )
                    nc.vector.memset(vaug[nt][:, 0:1], 1.0)
                    nc.vector.tensor_copy(vaug[nt][:, 1 : 1 + C], psv[:])

                if nsub == 2:
                    # data-arrival order: sb0 pieces land first and cover
                    # kq-sb0 plus v tiles j=0,1 (columns 0:256)
                    emit_kq_group(0)
                    emit_v_tile(0)
                    emit_v_tile(1)
                    emit_kq_group(1)
                    nc.vector.tensor_scalar_add(ktd_t[:, ch * 512 : (ch + 1) * 512], ps[:], aux_t[:, 0:1])
                    emit_v_tile(2)
                    emit_v_tile(3)
                else:
                    emit_kq_group(0)
                    nc.vector.tensor_scalar_add(ktd_t[:, ch * 512 : (ch + 1) * 512], ps[:], aux_t[:, 0:1])
                    for j in range(4):
                        emit_v_tile(j)
                # qT-gen (duplicated across partition halves) for query slab
                # qh = ch-1, read from the previous iteration's xc chunk
                # (x[b] is host-rotated so queries are chunks 0..3)
                if 1 <= ch <= NQ // 512:
                    qh = ch - 1
                    xq_src = xc_tiles[qh]
                    ps = ps_kq_pool.tile([P, 512], F32, name="pskq2", tag="kq")
                    for ks in range(CT):
                        nc.tensor.matmul(
                            ps[:],
                            wqq_t[:, ks * P : (ks + 1) * P],
                            xq_src[:, ks * 512 : (ks + 1) * 512],
                            start=(ks == 0),
                            stop=(ks == CT - 1),
                        )
                    nc.vector.tensor_scalar_add(qtd_t[:, qh * 512 : (qh + 1) * 512], ps[:], aux_t[:, 1:2])
                for g in E0_AFTER_CH.get(ch, []):
                    emit_e_quad(0, g)

        # ---- phase C: attention, software-pipelined query chunks of 256 ----
        with tc.tile_pool(name="psu", bufs=1, space="PSUM") as ps_u_pool, \
             tc.tile_pool(name="pst", bufs=2, space="PSUM") as ps_t_pool:
            def emit_u_slice(qc, qt, kt, psa, psb):
                g, j = divmod(kt, 4)
                jp = EPOS[j]
                lhs = pt_cur[g][:, jp * QC + qt * P : jp * QC + (qt + 1) * P]
                nc.tensor.matmul(psa[:], lhs, vaug[kt][:, 0:257], start=(kt == 0), stop=(kt == NT - 1))
                nc.tensor.matmul(psb[:], lhs, vaug[kt][:, 257:513], start=(kt == 0), stop=(kt == NT - 1))

            def emit_u_epilogue(qc, qt, psa, psb, us_ts):
                inv = small_pool.tile([P, 1], F32, name="inv", tag="inv")
                nc.vector.reciprocal(inv[:], psa[:, 0:1])
                us_t = us_pool.tile([P, C], BF16, name=f"us{qt}", tag=f"us{qt}")
                nc.vector.tensor_scalar_mul(us_t[:, 0:256], psa[:, 1:257], inv[:])
                nc.vector.tensor_scalar_mul(us_t[:, 256:512], psb[:, 0:256], inv[:])
                us_ts.append(us_t)

            last_y2 = {}

            def emit_tail_ct(qc, ct, us_ts):
                """Transpose + gamma/residual + store for one c-tile of chunk qc.
                For the final chunk the stores are batched in c-tile pairs:
                the four per-tile stores would otherwise serialize their
                ~1.4us DMA fixed cost into the kernel tail."""
                q0 = qc * QC
                last = qc == QCH - 1
                pst = ps_t_pool.tile([P, QC], BF16, name="pst", tag="t")
                for qt in range(QT):
                    nc.tensor.transpose(
                        pst[:, qt * P : (qt + 1) * P],
                        us_ts[qt][:, ct * P : (ct + 1) * P],
                        ident[:],
                    )
                xr = xr_pool.tile([P, QC], F32, name="xr", tag="xr")
                # final chunk: residual loads go on the scalar queue so they
                # don't queue behind the tail y-stores on sync
                (nc.scalar if last else nc.sync).dma_start(
                    xr[:], xqf_d[ct * P : (ct + 1) * P, q0 : q0 + QC]
                )
                if last:
                    if ct % 2 == 0:
                        last_y2[ct // 2] = y_pool.tile([P, 2 * QC], F32, name="y2", tag="y2", bufs=2)
                    yt = last_y2[ct // 2][:, (ct % 2) * QC : (ct % 2 + 1) * QC]
                else:
                    yt = y_pool.tile([P, QC], F32, name="yt", tag="y")[:]
                nc.scalar.activation(
                    yt, pst[:], AF.Identity,
                    bias=aux_t[:, 3 + ct : 4 + ct], scale=aux_t[:, 2:3],
                )
                nc.vector.tensor_add(yt, yt, xr[:])
                if last:
                    if ct % 2 == 1:
                        pair = ct // 2
                        nc.sync.dma_start(
                            y_d[pair * 2 * P : (pair + 1) * 2 * P, q0 : q0 + QC].rearrange(
                                "(ct p) q -> p ct q", ct=2
                            ),
                            last_y2[pair][:].rearrange("p (ct q) -> p ct q", ct=2),
                        )
                else:
                    nc.sync.dma_start(y_d[ct * P : (ct + 1) * P, q0 : q0 + QC], yt)

            prev_us = None  # us tiles of chunk c-1
            for qc in range(QCH):
                for g in range(NG):
                    pt_cur[g] = pt_next[g]
                    pt_next[g] = None
                us_ts = []
                psu = []
                for qt in range(QT):
                    psa = ps_u_pool.tile([P, 257], F32, name=f"psua{qt}", tag=f"ua{qt}")
                    psb = ps_u_pool.tile([P, 256], F32, name=f"psub{qt}", tag=f"ub{qt}")
                    psu.append((psa, psb))
                for i in range(2 * NT):  # 64 U slices
                    qt, kt = divmod(i, NT)
                    # interleave: one E^T quad for chunk qc+1 every 8 slices
                    if i % 8 == 4 and qc + 1 < QCH:
                        emit_e_quad(qc + 1, i // 8)
                    # interleave: tail work for chunk qc-1 during early slices
                    if prev_us is not None and i in (2, 10, 18, 26):
                        emit_tail_ct(qc - 1, i // 8, prev_us)
                    emit_u_slice(qc, qt, kt, *psu[qt])
                    if kt == NT - 1:
                        emit_u_epilogue(qc, qt, *psu[qt], us_ts)
                prev_us = us_ts
            for ct in range(CT):
                emit_tail_ct(QCH - 1, ct, prev_us)


def kernel(x, wq, bq, wk, bk, wv, bv, gamma, **_ignored):
    global _CACHED_NC
    x = np.ascontiguousarray(np.asarray(x, dtype=np.float32))
    wq = np.asarray(wq, dtype=np.float32)
    wk = np.asarray(wk, dtype=np.float32)
    wv = np.ascontiguousarray(np.asarray(wv, dtype=np.float32))
    bq = np.asarray(bq, dtype=np.float32)
    bk = np.asarray(bk, dtype=np.float32)
    bv = np.asarray(bv, dtype=np.float32)
    gamma = np.asarray(gamma, dtype=np.float32).reshape(())

    B = x.shape[0]
    assert x.shape == (B, C, 64, 64) and B * 2 == 8

    wkk = np.ascontiguousarray(np.concatenate([wk, wk], axis=1))  # [C, 128]
    wqq = np.ascontiguousarray(np.concatenate([wq, wq], axis=1))  # [C, 128]
    aux = np.zeros((P, 8), dtype=np.float32)
    aux[:, 0] = np.concatenate([bk, bk])
    aux[:, 1] = np.concatenate([bq, bq])
    aux[:, 2] = gamma
    gbv = gamma * bv
    for ct in range(CT):
        aux[:, 3 + ct] = gbv[ct * P : (ct + 1) * P]

    if _CACHED_NC is None:
        _CACHED_NC = _build_kernel()
    nc = _CACHED_NC

    xf = x.reshape(B, C, N)
    in_maps = []
    for core in range(8):
        b, h = divmod(core, 2)
        xqh = np.ascontiguousarray(xf[b][:, h * NQ : (h + 1) * NQ])
        xrot = np.ascontiguousarray(np.roll(xf[b], -h * NQ, axis=1))
        in_maps.append(
            {
                "xkv": xrot,
                "xqf": xqh,
                "wkk": wkk,
                "wqq": wqq,
                "wv": wv,
                "aux": aux,
            }
        )

    res = bass_utils.run_bass_kernel_spmd(nc, in_maps, core_ids=list(range(8)))
    if res.exec_time_ns is not None:
        print(f"HW exec time: {res.exec_time_ns} ns", flush=True)
        if res.instructions_and_trace is not None:
            print(f"trace: {res.instructions_and_trace[1]}", flush=True)

    y = np.empty((B, C, N), dtype=np.float32)
    for core in range(8):
        b, h = divmod(core, 2)
        y[b][:, h * NQ : (h + 1) * NQ] = res.results[core]["y"]
    return y.reshape(B, C, 64, 64)
